# revision 12
# baseline (speedup 1.0000x reference)
"""Trainium2 Bass kernel v3 for Conv2Demod — Winograd F(2x2,3x3), bf16 PE.

Per-sample computation restructured as Winograd:
    out = A^T [ (G w G^T * (1+s_i) * d_o) elemwise (B^T d B) ] A
  - Host precomputes wg[u,v,i,o] = (G W G^T); the per-channel modulation
    (1+s_i) commutes with G and is applied on-device as a per-partition
    scale on GpSimd.  Demod d_o is folded into the PSUM eviction scale.
  - MAC count drops 2.25x vs direct conv: 16 (u,v) matmuls over 32x32
    output tiles instead of 9 taps over 64x64 pixels.
  - Input transform (B^T d B) and output transform (A^T M A) run on the
    DVE as tensor_tensor add/sub chains, overlapping the PE.  The padded
    image is stored x-DE-INTERLEAVED (even columns then odd columns) so
    every transform access has unit innermost stride, which qualifies the
    bf16 ops for the DVE 2x performance mode.

Pipeline per core (one sample): two y-halves of 16 tile-rows each;
per half: DVE transforms the image into V[it][uv, 512 tiles] (bf16),
then for each o-tile the PE runs 2 phases x 8 uv x 4 i-tile
accumulating matmuls (8 PSUM banks), ACT evicts with demod scale to
bf16 M, DVE applies the output transform into f32 osb, DMA to DRAM.
Sharding: one sample per NeuronCore, 8 cores data-parallel.
"""

import contextlib

import numpy as np
import ml_dtypes

import concourse.bacc as bacc
import concourse.mybir as mybir
import concourse.tile as tile
from concourse.bass_utils import run_bass_kernel_spmd

P = 128
CIN = 512
COUT = 512
H = W = 64
NI = CIN // P
NO = COUT // P
T = H // 2          # 32 tile rows/cols
TH = T // 2         # 16 tile rows per half
NPIX = TH * T       # 512 tiles per half
HP = H + 2
WP = W + 2          # 66 = 33 even + 33 odd x-positions
XE = WP // 2        # 33
EPS = 1e-8
N_CORES = 8

F32 = mybir.dt.float32
BF16 = mybir.dt.bfloat16
AF = mybir.ActivationFunctionType
ALU = mybir.AluOpType
_nullcm = contextlib.nullcontext


def build_nc(loop_n=None):
    nc = bacc.Bacc("TRN2", target_bir_lowering=False, debug=False)

    img = nc.dram_tensor("img", [NI, P, HP, WP], BF16, kind="ExternalInput").ap()
    s_in = nc.dram_tensor("s", [CIN], F32, kind="ExternalInput").ap()
    wg = nc.dram_tensor("wg", [NI, P, 16 * COUT], BF16, kind="ExternalInput").ap()
    at = nc.dram_tensor("at", [CIN, COUT], BF16, kind="ExternalInput").ap()
    out = nc.dram_tensor("out", [COUT, H, W], F32, kind="ExternalOutput").ap()

    with tile.TileContext(nc) as tc:
        with (
            tc.tile_pool(name="const", bufs=1) as cpool,
            tc.tile_pool(name="persist", bufs=1) as ppool,
            tc.tile_pool(name="imgh", bufs=5) as imghp,
            tc.tile_pool(name="v1", bufs=2) as v1p,
            tc.tile_pool(name="msb", bufs=1) as msbp,
            tc.tile_pool(name="psb", bufs=1) as psbp,
            tc.tile_pool(name="osb", bufs=2) as osbp,
            tc.tile_pool(name="psum", bufs=8, space="PSUM") as psum_pool,
        ):
            with (tc.For_i(0, loop_n, 1) if loop_n else _nullcm()):
                # ---- half-0 image first so the DVE transform starts
                # immediately instead of queuing behind the 8.4MB weight DMA
                imgh_pre = []
                for it in range(NI):
                    imgh = imghp.tile([P, 17, 2, WP], BF16, tag="imgh")
                    imgh_pre.append(imgh)
                    nc.sync.dma_start(
                        imgh[:].rearrange("p a b x -> p (a b) x"),
                        img[it][:, 0:34, :],
                    )
                # ---- s-derived scalars -------------------------------------
                sraw = cpool.tile([P, NI, 2], F32, tag="sraw")
                for c in range(2):
                    nc.sync.dma_start(
                        sraw[:, :, c], s_in.rearrange("(t p) -> p t", p=P)
                    )
                smod = cpool.tile([P, NI, 2], F32, tag="smod")  # 1 + s
                nc.scalar.activation(smod[:], sraw[:], AF.Copy, bias=1.0)
                tsq = cpool.tile([P, NI, 2], BF16, tag="tsq")   # (1 + s)^2
                nc.scalar.square(tsq[:], smod[:])

                # ---- demod d[o] = 1/sqrt(A_T.T @ tsq + eps) ----------------
                at_sb = ppool.tile([P, NI, COUT], BF16, tag="at_sb")
                nc.sync.dma_start(
                    at_sb[:], at.rearrange("(t p) o -> p t o", p=P)
                )
                dsb = cpool.tile([P, NO], F32, tag="dsb")
                dtmp = cpool.tile([P, NO], F32, tag="dtmp")
                epst = cpool.tile([P, 1], F32, tag="epst")
                nc.vector.memset(epst[:], EPS)
                for ot in range(NO):
                    o0 = ot * P
                    psd = psum_pool.tile([P, NPIX], F32, tag="ps")
                    for it in range(NI):
                        nc.tensor.matmul(
                            psd[:, 0:2],
                            at_sb[:, it, o0 : o0 + P],
                            tsq[:, it, :],
                            start=(it == 0),
                            stop=(it == NI - 1),
                        )
                    nc.scalar.activation(
                        dtmp[:, ot : ot + 1], psd[:, 0:1], AF.Sqrt, bias=epst[:]
                    )
                nc.vector.reciprocal(dsb[:], dtmp[:])

                # ---- transformed weights (unmodulated; the (1+s) scale is
                # applied to the image on the DVE at 4x instead) -------------
                wmod = []
                for it in range(NI):
                    wm = ppool.tile([P, 16, COUT], BF16, tag=f"wmod{it}")
                    wmod.append(wm)
                    nc.sync.dma_start(
                        wm[:].rearrange("p a b -> p (a b)"), wg[it]
                    )

                # ---- per half: input transform, matmuls, output transform --
                vt = []
                for it in range(NI):
                    v = ppool.tile([P, 16, TH, T], BF16, tag=f"vt{it}")
                    vt.append(v)

                def tt(o, a, b, op):
                    nc.vector.tensor_tensor(o, a, b, op)

                for h in range(2):
                    y0 = 2 * TH * h  # padded-image row base (=32h)
                    # -- input transform (u-major so PE phase A starts after
                    #    u=0,1 are done instead of after all four i-tiles) --
                    if h == 0:
                        imghs = imgh_pre
                    else:
                        imghs = []
                        for it in range(NI):
                            imgh = imghp.tile([P, 17, 2, WP], BF16, tag="imgh")
                            imghs.append(imgh)
                            nc.sync.dma_start(
                                imgh[:].rearrange("p a b x -> p (a b) x"),
                                img[it][:, y0 : y0 + 34, :],
                            )
                    # per-channel modulation on the image (4x tensor_scalar)
                    for it in range(NI):
                        nc.vector.tensor_scalar_mul(
                            imghs[it][:].rearrange("p a b x -> p (a b x)"),
                            imghs[it][:].rearrange("p a b x -> p (a b x)"),
                            smod[:, it, 0:1],
                        )
                    for u in range(4):
                        for it in range(NI):
                            imgh = imghs[it]
                            d0 = imgh[:, 0:TH, 0, :]
                            d1 = imgh[:, 0:TH, 1, :]
                            d2 = imgh[:, 1 : TH + 1, 0, :]
                            d3 = imgh[:, 1 : TH + 1, 1, :]
                            # BT: u0=d0-d2, u1=d1+d2, u2=d2-d1, u3=d1-d3
                            upat = [
                                (d0, d2, ALU.subtract),
                                (d1, d2, ALU.add),
                                (d2, d1, ALU.subtract),
                                (d1, d3, ALU.subtract),
                            ][u]
                            # v1 keeps the de-interleaved x layout
                            v1 = v1p.tile([P, TH, WP], BF16, tag="v1")
                            tt(v1[:], upat[0], upat[1], upat[2])
                            x0 = v1[:, :, 0:T]            # even x: 0..62
                            x2 = v1[:, :, 1 : T + 1]      # even x: 2..64
                            x1 = v1[:, :, XE : XE + T]    # odd x: 1..63
                            x3 = v1[:, :, XE + 1 : XE + T + 1]  # odd: 3..65
                            vpat = [
                                (x0, x2, ALU.subtract),
                                (x1, x2, ALU.add),
                                (x2, x1, ALU.subtract),
                                (x1, x3, ALU.subtract),
                            ]
                            for v, (xa, xb, xop) in enumerate(vpat):
                                tt(vt[it][:, u * 4 + v], xa, xb, xop)

                    # -- matmuls + output transform per o-tile --------------
                    for ot in range(NO):
                        o0 = ot * P
                        msb = msbp.tile([P, 16, NPIX], BF16, tag="msb")
                        for ph in range(2):
                            pss = []
                            for k in range(8):
                                ps = psum_pool.tile([P, NPIX], F32, tag="ps")
                                pss.append(ps)
                            for it in range(NI):
                                for k in range(8):
                                    uv = ph * 8 + k
                                    nc.tensor.matmul(
                                        pss[k][:],
                                        wmod[it][:, uv, o0 : o0 + P],
                                        vt[it][:, uv].rearrange(
                                            "p t c -> p (t c)"),
                                        start=(it == 0),
                                        stop=(it == NI - 1),
                                    )
                            for k in range(8):
                                nc.scalar.activation(
                                    msb[:, ph * 8 + k, :], pss[k][:],
                                    AF.Copy, scale=dsb[:, ot : ot + 1],
                                )
                        # output transform: AT = [[1,1,1,0],[0,1,-1,-1]]
                        psb = psbp.tile([P, 8, NPIX], BF16, tag="psb")
                        M = [msb[:, uv, :] for uv in range(16)]
                        for v in range(4):
                            p0 = psb[:, v, :]
                            p1 = psb[:, 4 + v, :]
                            tt(p0, M[0 * 4 + v], M[1 * 4 + v], ALU.add)
                            tt(p0, p0, M[2 * 4 + v], ALU.add)
                            tt(p1, M[1 * 4 + v], M[2 * 4 + v], ALU.subtract)
                            tt(p1, p1, M[3 * 4 + v], ALU.subtract)
                        osb = osbp.tile([P, TH, 2, T, 2], F32, tag="osb")
                        for a in range(2):
                            pa = [
                                psb[:, a * 4 + v, :].rearrange(
                                    "p (t c) -> p t c", c=T)
                                for v in range(4)
                            ]
                            yr = [osb[:, :, a, :, b] for b in range(2)]
                            tt(yr[0], pa[0], pa[1], ALU.add)
                            tt(yr[0], yr[0], pa[2], ALU.add)
                            tt(yr[1], pa[1], pa[2], ALU.subtract)
                            tt(yr[1], yr[1], pa[3], ALU.subtract)
                        nc.sync.dma_start(
                            out[o0 : o0 + P, 2 * TH * h : 2 * TH * (h + 1), :],
                            osb[:].rearrange("p t a c b -> p (t a) (c b)"),
                        )
    nc.compile()
    return nc


_NC_CACHE = None


def _get_nc():
    global _NC_CACHE
    if _NC_CACHE is None:
        _NC_CACHE = build_nc()
    return _NC_CACHE


_G = np.array(
    [[1, 0, 0], [0.5, 0.5, 0.5], [0.5, -0.5, 0.5], [0, 0, 1]], np.float64
)


def make_in_maps(img, s, weight):
    img = np.asarray(img, dtype=np.float32)
    s = np.ascontiguousarray(np.asarray(s, dtype=np.float32))
    weight = np.asarray(weight, dtype=np.float32)
    # zero-pad, then DE-INTERLEAVE x: [even cols | odd cols]
    imgp = np.zeros((img.shape[0], NI, P, HP, WP), dtype=np.float32)
    imgp[:, :, :, 1 : H + 1, 1 : W + 1] = img.reshape(-1, NI, P, H, W)
    imgd = np.concatenate(
        [imgp[..., 0::2], imgp[..., 1::2]], axis=-1
    ).astype(ml_dtypes.bfloat16)
    # wg[u,v,i,o] = (G W G^T)[o,i,u,v] -> [NI, P, (u,v,o)]
    wgf = np.einsum("ua,oiab,vb->uvio", _G, weight.astype(np.float64), _G)
    wgt = np.ascontiguousarray(
        wgf.transpose(2, 0, 1, 3)          # [i, u, v, o]
        .reshape(NI, P, 16 * COUT)
        .astype(ml_dtypes.bfloat16)
    )
    at = np.ascontiguousarray(
        (weight.astype(np.float64) ** 2)
        .sum(axis=(2, 3))
        .T.astype(ml_dtypes.bfloat16)
    )
    return [
        {"img": imgd[b], "s": s[b], "wg": wgt, "at": at} for b in range(N_CORES)
    ]


def kernel(img, s, weight):
    nc = _get_nc()
    in_maps = make_in_maps(img, s, weight)
    res = run_bass_kernel_spmd(nc, in_maps, list(range(N_CORES)))
    return np.stack([res.results[b]["out"] for b in range(N_CORES)], axis=0)


# revision 15
# speedup vs baseline: 1.1313x; 1.1313x over previous
"""Trainium2 Bass kernel v3 for Conv2Demod — Winograd F(2x2,3x3), bf16 PE.

Per-sample computation restructured as Winograd:
    out = A^T [ (G w G^T * (1+s_i) * d_o) elemwise (B^T d B) ] A
  - Host precomputes wg[u,v,i,o] = (G W G^T); the per-channel modulation
    (1+s_i) commutes with G and is applied on-device as a per-partition
    scale on GpSimd.  Demod d_o is folded into the PSUM eviction scale.
  - MAC count drops 2.25x vs direct conv: 16 (u,v) matmuls over 32x32
    output tiles instead of 9 taps over 64x64 pixels.
  - Input transform (B^T d B) and output transform (A^T M A) run on the
    DVE as tensor_tensor add/sub chains, overlapping the PE.  The padded
    image is stored x-DE-INTERLEAVED (even columns then odd columns) so
    every transform access has unit innermost stride, which qualifies the
    bf16 ops for the DVE 2x performance mode.

Pipeline per core (one sample): two y-halves of 16 tile-rows each;
per half: DVE transforms the image into V[it][uv, 512 tiles] (bf16),
then for each o-tile the PE runs 2 phases x 8 uv x 4 i-tile
accumulating matmuls (8 PSUM banks), ACT evicts with demod scale to
bf16 M, DVE applies the output transform into f32 osb, DMA to DRAM.
Sharding: one sample per NeuronCore, 8 cores data-parallel.
"""

import contextlib

import numpy as np
import ml_dtypes

import concourse.bacc as bacc
import concourse.mybir as mybir
import concourse.tile as tile
from concourse.bass_utils import run_bass_kernel_spmd

P = 128
CIN = 512
COUT = 512
H = W = 64
NI = CIN // P
NO = COUT // P
T = H // 2          # 32 tile rows/cols
TH = T // 2         # 16 tile rows per half
NPIX = TH * T       # 512 tiles per half
HP = H + 2
WP = W + 2          # 66 = 33 even + 33 odd x-positions
XE = WP // 2        # 33
EPS = 1e-8
N_CORES = 8

F32 = mybir.dt.float32
BF16 = mybir.dt.bfloat16
AF = mybir.ActivationFunctionType
ALU = mybir.AluOpType
_nullcm = contextlib.nullcontext


def build_nc(loop_n=None):
    nc = bacc.Bacc("TRN2", target_bir_lowering=False, debug=False)

    img = nc.dram_tensor("img", [NI, P, HP, WP], BF16, kind="ExternalInput").ap()
    s_in = nc.dram_tensor("s", [CIN], F32, kind="ExternalInput").ap()
    wg = nc.dram_tensor("wg", [NI, P, 16 * COUT], BF16, kind="ExternalInput").ap()
    at = nc.dram_tensor("at", [CIN, COUT], BF16, kind="ExternalInput").ap()
    out = nc.dram_tensor("out", [COUT, H, W], F32, kind="ExternalOutput").ap()

    with tile.TileContext(nc) as tc:
        with (
            tc.tile_pool(name="const", bufs=1) as cpool,
            tc.tile_pool(name="persist", bufs=1) as ppool,
            tc.tile_pool(name="imgh", bufs=5) as imghp,
            tc.tile_pool(name="v1", bufs=2) as v1p,
            tc.tile_pool(name="msb", bufs=1) as msbp,
            tc.tile_pool(name="psb", bufs=1) as psbp,
            tc.tile_pool(name="osb", bufs=2) as osbp,
            tc.tile_pool(name="psum", bufs=8, space="PSUM") as psum_pool,
        ):
            with (tc.For_i(0, loop_n, 1) if loop_n else _nullcm()):
                # ---- half-0 image first so the DVE transform starts
                # immediately instead of queuing behind the 8.4MB weight DMA
                imgh_pre = []
                for it in range(NI):
                    imgh = imghp.tile([P, 17, 2, WP], BF16, tag="imgh")
                    imgh_pre.append(imgh)
                    nc.sync.dma_start(
                        imgh[:].rearrange("p a b x -> p (a b) x"),
                        img[it][:, 0:34, :],
                    )
                # ---- s-derived scalars -------------------------------------
                sraw = cpool.tile([P, NI, 2], F32, tag="sraw")
                for c in range(2):
                    nc.sync.dma_start(
                        sraw[:, :, c], s_in.rearrange("(t p) -> p t", p=P)
                    )
                smod = cpool.tile([P, NI, 2], F32, tag="smod")  # 1 + s
                nc.scalar.activation(smod[:], sraw[:], AF.Copy, bias=1.0)
                tsq = cpool.tile([P, NI, 2], BF16, tag="tsq")   # (1 + s)^2
                nc.scalar.square(tsq[:], smod[:])

                # ---- demod d[o] = 1/sqrt(A_T.T @ tsq + eps) ----------------
                at_sb = ppool.tile([P, NI, COUT], BF16, tag="at_sb")
                nc.sync.dma_start(
                    at_sb[:], at.rearrange("(t p) o -> p t o", p=P)
                )
                dsb = cpool.tile([P, NO], F32, tag="dsb")
                dtmp = cpool.tile([P, NO], F32, tag="dtmp")
                epst = cpool.tile([P, 1], F32, tag="epst")
                nc.vector.memset(epst[:], EPS)
                for ot in range(NO):
                    o0 = ot * P
                    psd = psum_pool.tile([P, NPIX], F32, tag="ps")
                    for it in range(NI):
                        nc.tensor.matmul(
                            psd[:, 0:2],
                            at_sb[:, it, o0 : o0 + P],
                            tsq[:, it, :],
                            start=(it == 0),
                            stop=(it == NI - 1),
                        )
                    nc.scalar.activation(
                        dtmp[:, ot : ot + 1], psd[:, 0:1], AF.Sqrt, bias=epst[:]
                    )
                nc.vector.reciprocal(dsb[:], dtmp[:])

                # ---- transformed weights (unmodulated; the (1+s) scale is
                # applied to the image on the DVE at 4x instead) -------------
                wmod = []
                for it in range(NI):
                    wm = ppool.tile([P, 16, COUT], BF16, tag=f"wmod{it}")
                    wmod.append(wm)
                    nc.sync.dma_start(
                        wm[:].rearrange("p a b -> p (a b)"), wg[it]
                    )

                # ---- per half: input transform, matmuls, output transform --
                vt = []
                for it in range(NI):
                    v = ppool.tile([P, 16, TH, T], BF16, tag=f"vt{it}")
                    vt.append(v)

                def tt(o, a, b, op):
                    nc.vector.tensor_tensor(o, a, b, op)

                for h in range(2):
                    y0 = 2 * TH * h  # padded-image row base (=32h)
                    # -- input transform (u-major so PE phase A starts after
                    #    u=0,1 are done instead of after all four i-tiles) --
                    if h == 0:
                        imghs = imgh_pre
                    else:
                        imghs = []
                        for it in range(NI):
                            imgh = imghp.tile([P, 17, 2, WP], BF16, tag="imgh")
                            imghs.append(imgh)
                            nc.sync.dma_start(
                                imgh[:].rearrange("p a b x -> p (a b) x"),
                                img[it][:, y0 : y0 + 34, :],
                            )
                    # per-channel modulation on the image (4x tensor_scalar)
                    for it in range(NI):
                        nc.vector.tensor_scalar_mul(
                            imghs[it][:].rearrange("p a b x -> p (a b x)"),
                            imghs[it][:].rearrange("p a b x -> p (a b x)"),
                            smod[:, it, 0:1],
                        )
                    for u in range(4):
                        for it in range(NI):
                            imgh = imghs[it]
                            d0 = imgh[:, 0:TH, 0, :]
                            d1 = imgh[:, 0:TH, 1, :]
                            d2 = imgh[:, 1 : TH + 1, 0, :]
                            d3 = imgh[:, 1 : TH + 1, 1, :]
                            # BT: u0=d0-d2, u1=d1+d2, u2=d2-d1, u3=d1-d3
                            upat = [
                                (d0, d2, ALU.subtract),
                                (d1, d2, ALU.add),
                                (d2, d1, ALU.subtract),
                                (d1, d3, ALU.subtract),
                            ][u]
                            # v1 keeps the de-interleaved x layout
                            v1 = v1p.tile([P, TH, WP], BF16, tag="v1")
                            tt(v1[:], upat[0], upat[1], upat[2])
                            x0 = v1[:, :, 0:T]            # even x: 0..62
                            x2 = v1[:, :, 1 : T + 1]      # even x: 2..64
                            x1 = v1[:, :, XE : XE + T]    # odd x: 1..63
                            x3 = v1[:, :, XE + 1 : XE + T + 1]  # odd: 3..65
                            vpat = [
                                (x0, x2, ALU.subtract),
                                (x1, x2, ALU.add),
                                (x2, x1, ALU.subtract),
                                (x1, x3, ALU.subtract),
                            ]
                            for v, (xa, xb, xop) in enumerate(vpat):
                                tt(vt[it][:, u * 4 + v], xa, xb, xop)

                    # -- matmuls + output transform per o-tile --------------
                    for ot in range(NO):
                        o0 = ot * P
                        msb = msbp.tile([P, 16, NPIX], BF16, tag="msb")
                        psb = psbp.tile([P, 8, NPIX], BF16, tag="psb")
                        M = [msb[:, uv, :] for uv in range(16)]
                        # v-split phases: pass1 for a v needs all four u's of
                        # that v, so it runs as soon as its phase evicts --
                        # half the output transform overlaps the next phase.
                        for vset in ((0, 1), (2, 3)):
                            uvs = [u * 4 + v for u in range(4) for v in vset]
                            pss = []
                            for k in range(8):
                                ps = psum_pool.tile([P, NPIX], F32, tag="ps")
                                pss.append(ps)
                            # uv-major: 4 consecutive MMs finish a bank, so
                            # its eviction overlaps the next uv's matmuls
                            for k, uv in enumerate(uvs):
                                for it in range(NI):
                                    nc.tensor.matmul(
                                        pss[k][:],
                                        wmod[it][:, uv, o0 : o0 + P],
                                        vt[it][:, uv].rearrange(
                                            "p t c -> p (t c)"),
                                        start=(it == 0),
                                        stop=(it == NI - 1),
                                    )
                            for k, uv in enumerate(uvs):
                                nc.scalar.activation(
                                    msb[:, uv, :], pss[k][:],
                                    AF.Copy, scale=dsb[:, ot : ot + 1],
                                )
                            # output transform pass1 for this phase's v's
                            # AT = [[1,1,1,0],[0,1,-1,-1]]
                            for v in vset:
                                p0 = psb[:, v, :]
                                p1 = psb[:, 4 + v, :]
                                tt(p0, M[0 * 4 + v], M[1 * 4 + v], ALU.add)
                                tt(p0, p0, M[2 * 4 + v], ALU.add)
                                tt(p1, M[1 * 4 + v], M[2 * 4 + v],
                                   ALU.subtract)
                                tt(p1, p1, M[3 * 4 + v], ALU.subtract)
                        osb = osbp.tile([P, TH, 2, T, 2], F32, tag="osb")
                        for a in range(2):
                            pa = [
                                psb[:, a * 4 + v, :].rearrange(
                                    "p (t c) -> p t c", c=T)
                                for v in range(4)
                            ]
                            yr = [osb[:, :, a, :, b] for b in range(2)]
                            tt(yr[0], pa[0], pa[1], ALU.add)
                            tt(yr[0], yr[0], pa[2], ALU.add)
                            tt(yr[1], pa[1], pa[2], ALU.subtract)
                            tt(yr[1], yr[1], pa[3], ALU.subtract)
                        nc.sync.dma_start(
                            out[o0 : o0 + P, 2 * TH * h : 2 * TH * (h + 1), :],
                            osb[:].rearrange("p t a c b -> p (t a) (c b)"),
                        )
    nc.compile()
    return nc


_NC_CACHE = None


def _get_nc():
    global _NC_CACHE
    if _NC_CACHE is None:
        _NC_CACHE = build_nc()
    return _NC_CACHE


_G = np.array(
    [[1, 0, 0], [0.5, 0.5, 0.5], [0.5, -0.5, 0.5], [0, 0, 1]], np.float64
)


def make_in_maps(img, s, weight):
    img = np.asarray(img, dtype=np.float32)
    s = np.ascontiguousarray(np.asarray(s, dtype=np.float32))
    weight = np.asarray(weight, dtype=np.float32)
    # zero-pad, then DE-INTERLEAVE x: [even cols | odd cols]
    imgp = np.zeros((img.shape[0], NI, P, HP, WP), dtype=np.float32)
    imgp[:, :, :, 1 : H + 1, 1 : W + 1] = img.reshape(-1, NI, P, H, W)
    imgd = np.concatenate(
        [imgp[..., 0::2], imgp[..., 1::2]], axis=-1
    ).astype(ml_dtypes.bfloat16)
    # wg[u,v,i,o] = (G W G^T)[o,i,u,v] -> [NI, P, (u,v,o)]
    wgf = np.einsum("ua,oiab,vb->uvio", _G, weight.astype(np.float64), _G)
    wgt = np.ascontiguousarray(
        wgf.transpose(2, 0, 1, 3)          # [i, u, v, o]
        .reshape(NI, P, 16 * COUT)
        .astype(ml_dtypes.bfloat16)
    )
    at = np.ascontiguousarray(
        (weight.astype(np.float64) ** 2)
        .sum(axis=(2, 3))
        .T.astype(ml_dtypes.bfloat16)
    )
    return [
        {"img": imgd[b], "s": s[b], "wg": wgt, "at": at} for b in range(N_CORES)
    ]


def kernel(img, s, weight):
    nc = _get_nc()
    in_maps = make_in_maps(img, s, weight)
    res = run_bass_kernel_spmd(nc, in_maps, list(range(N_CORES)))
    return np.stack([res.results[b]["out"] for b in range(N_CORES)], axis=0)


# revision 18
# speedup vs baseline: 1.2712x; 1.1237x over previous
"""Trainium2 Bass kernel v3 for Conv2Demod — Winograd F(2x2,3x3), bf16 PE.

Per-sample computation restructured as Winograd:
    out = A^T [ (G w G^T * (1+s_i) * d_o) elemwise (B^T d B) ] A
  - Host precomputes wg[u,v,i,o] = (G W G^T); the per-channel modulation
    (1+s_i) commutes with G and is applied on-device as a per-partition
    scale on GpSimd.  Demod d_o is folded into the PSUM eviction scale.
  - MAC count drops 2.25x vs direct conv: 16 (u,v) matmuls over 32x32
    output tiles instead of 9 taps over 64x64 pixels.
  - Input transform (B^T d B) and output transform (A^T M A) run on the
    DVE as tensor_tensor add/sub chains, overlapping the PE.  The padded
    image is stored x-DE-INTERLEAVED (even columns then odd columns) so
    every transform access has unit innermost stride, which qualifies the
    bf16 ops for the DVE 2x performance mode.

Pipeline per core (one sample): two y-halves of 16 tile-rows each;
per half: DVE transforms the image into V[it][uv, 512 tiles] (bf16),
then for each o-tile the PE runs 2 phases x 8 uv x 4 i-tile
accumulating matmuls (8 PSUM banks), ACT evicts with demod scale to
bf16 M, DVE applies the output transform into f32 osb, DMA to DRAM.
Sharding: one sample per NeuronCore, 8 cores data-parallel.
"""

import contextlib

import numpy as np
import ml_dtypes

import concourse.bacc as bacc
import concourse.mybir as mybir
import concourse.tile as tile
from concourse.bass_utils import run_bass_kernel_spmd

P = 128
CIN = 512
COUT = 512
H = W = 64
NI = CIN // P
NO = COUT // P
T = H // 2          # 32 tile rows/cols
TH = T // 2         # 16 tile rows per half
NPIX = TH * T       # 512 tiles per half
HP = H + 2
WP = W + 2          # 66 = 33 even + 33 odd x-positions
XE = WP // 2        # 33
EPS = 1e-8
N_CORES = 8

F32 = mybir.dt.float32
BF16 = mybir.dt.bfloat16
AF = mybir.ActivationFunctionType
ALU = mybir.AluOpType
_nullcm = contextlib.nullcontext


def build_nc(loop_n=None):
    nc = bacc.Bacc("TRN2", target_bir_lowering=False, debug=False)

    img = nc.dram_tensor("img", [NI, P, HP, WP], BF16, kind="ExternalInput").ap()
    s_in = nc.dram_tensor("s", [CIN], F32, kind="ExternalInput").ap()
    wg = nc.dram_tensor("wg", [NI, P, 16 * COUT], BF16, kind="ExternalInput").ap()
    at = nc.dram_tensor("at", [CIN, COUT], BF16, kind="ExternalInput").ap()
    out = nc.dram_tensor("out", [COUT, H, W], F32, kind="ExternalOutput").ap()

    with tile.TileContext(nc) as tc:
        with (
            tc.tile_pool(name="const", bufs=1) as cpool,
            tc.tile_pool(name="persist", bufs=1) as ppool,
            tc.tile_pool(name="imgh", bufs=5) as imghp,
            tc.tile_pool(name="v1", bufs=2) as v1p,
            tc.tile_pool(name="msb", bufs=1) as msbp,
            tc.tile_pool(name="psb", bufs=1) as psbp,
            tc.tile_pool(name="osb", bufs=2) as osbp,
            tc.tile_pool(name="psum", bufs=8, space="PSUM") as psum_pool,
        ):
            with (tc.For_i(0, loop_n, 1) if loop_n else _nullcm()):
                # ---- half-0 image first so the DVE transform starts
                # immediately instead of queuing behind the 8.4MB weight DMA
                imgh_pre = []
                for it in range(NI):
                    imgh = imghp.tile([P, 17, 2, WP], BF16, tag="imgh")
                    imgh_pre.append(imgh)
                    nc.sync.dma_start(
                        imgh[:].rearrange("p a b x -> p (a b) x"),
                        img[it][:, 0:34, :],
                    )
                # ---- s-derived scalars -------------------------------------
                sraw = cpool.tile([P, NI, 2], F32, tag="sraw")
                for c in range(2):
                    nc.sync.dma_start(
                        sraw[:, :, c], s_in.rearrange("(t p) -> p t", p=P)
                    )
                smod = cpool.tile([P, NI, 2], F32, tag="smod")  # 1 + s
                nc.scalar.activation(smod[:], sraw[:], AF.Copy, bias=1.0)
                tsq = cpool.tile([P, NI, 2], BF16, tag="tsq")   # (1 + s)^2
                nc.scalar.square(tsq[:], smod[:])

                # ---- demod d[o] = 1/sqrt(A_T.T @ tsq + eps) ----------------
                at_sb = ppool.tile([P, NI, COUT], BF16, tag="at_sb")
                nc.sync.dma_start(
                    at_sb[:], at.rearrange("(t p) o -> p t o", p=P)
                )
                dsb = cpool.tile([P, NO], F32, tag="dsb")
                dtmp = cpool.tile([P, NO], F32, tag="dtmp")
                epst = cpool.tile([P, 1], F32, tag="epst")
                nc.vector.memset(epst[:], EPS)
                for ot in range(NO):
                    o0 = ot * P
                    psd = psum_pool.tile([P, NPIX], F32, tag="ps")
                    for it in range(NI):
                        nc.tensor.matmul(
                            psd[:, 0:2],
                            at_sb[:, it, o0 : o0 + P],
                            tsq[:, it, :],
                            start=(it == 0),
                            stop=(it == NI - 1),
                        )
                    nc.scalar.activation(
                        dtmp[:, ot : ot + 1], psd[:, 0:1], AF.Sqrt, bias=epst[:]
                    )
                nc.vector.reciprocal(dsb[:], dtmp[:])

                # ---- transformed weights (unmodulated; the (1+s) scale is
                # applied to the image on the DVE at 4x instead) -------------
                wmod = []
                for it in range(NI):
                    wm = ppool.tile([P, 16, COUT], BF16, tag=f"wmod{it}")
                    wmod.append(wm)
                    nc.sync.dma_start(
                        wm[:].rearrange("p a b -> p (a b)"), wg[it]
                    )

                # ---- per half: input transform, matmuls, output transform --
                vt = []
                for it in range(NI):
                    v = ppool.tile([P, 16, TH, T], BF16, tag=f"vt{it}")
                    vt.append(v)

                def tt(o, a, b, op):
                    nc.vector.tensor_tensor(o, a, b, op)

                for h in range(2):
                    y0 = 2 * TH * h  # padded-image row base (=32h)
                    # -- input transform (u-major so PE phase A starts after
                    #    u=0,1 are done instead of after all four i-tiles) --
                    if h == 0:
                        imghs = imgh_pre
                    else:
                        imghs = []
                        for it in range(NI):
                            imgh = imghp.tile([P, 17, 2, WP], BF16, tag="imgh")
                            imghs.append(imgh)
                            nc.sync.dma_start(
                                imgh[:].rearrange("p a b x -> p (a b) x"),
                                img[it][:, y0 : y0 + 34, :],
                            )
                    # per-channel modulation on the image (4x tensor_scalar)
                    for it in range(NI):
                        nc.vector.tensor_scalar_mul(
                            imghs[it][:].rearrange("p a b x -> p (a b x)"),
                            imghs[it][:].rearrange("p a b x -> p (a b x)"),
                            smod[:, it, 0:1],
                        )
                    for u in range(4):
                        for it in range(NI):
                            imgh = imghs[it]
                            d0 = imgh[:, 0:TH, 0, :]
                            d1 = imgh[:, 0:TH, 1, :]
                            d2 = imgh[:, 1 : TH + 1, 0, :]
                            d3 = imgh[:, 1 : TH + 1, 1, :]
                            # BT: u0=d0-d2, u1=d1+d2, u2=d2-d1, u3=d1-d3
                            upat = [
                                (d0, d2, ALU.subtract),
                                (d1, d2, ALU.add),
                                (d2, d1, ALU.subtract),
                                (d1, d3, ALU.subtract),
                            ][u]
                            # v1 keeps the de-interleaved x layout
                            v1 = v1p.tile([P, TH, WP], BF16, tag="v1")
                            tt(v1[:], upat[0], upat[1], upat[2])
                            x0 = v1[:, :, 0:T]            # even x: 0..62
                            x2 = v1[:, :, 1 : T + 1]      # even x: 2..64
                            x1 = v1[:, :, XE : XE + T]    # odd x: 1..63
                            x3 = v1[:, :, XE + 1 : XE + T + 1]  # odd: 3..65
                            vpat = [
                                (x0, x2, ALU.subtract),
                                (x1, x2, ALU.add),
                                (x2, x1, ALU.subtract),
                                (x1, x3, ALU.subtract),
                            ]
                            for v, (xa, xb, xop) in enumerate(vpat):
                                tt(vt[it][:, u * 4 + v], xa, xb, xop)

                    # -- matmuls + output transform per o-tile --------------
                    for ot in range(NO):
                        o0 = ot * P
                        msb = msbp.tile([P, 16, NPIX], BF16, tag="msb")
                        psb = psbp.tile([P, 8, NPIX], BF16, tag="psb")
                        M = [msb[:, uv, :] for uv in range(16)]
                        # v-split phases: pass1 for a v needs all four u's of
                        # that v, so it runs as soon as its phase evicts --
                        # half the output transform overlaps the next phase.
                        for vset in ((0, 1), (2, 3)):
                            uvs = [u * 4 + v for u in range(4) for v in vset]
                            pss = []
                            for k in range(8):
                                ps = psum_pool.tile([P, NPIX], F32, tag="ps")
                                pss.append(ps)
                            # uv-major: 4 consecutive MMs finish a bank, so
                            # its eviction overlaps the next uv's matmuls
                            for k, uv in enumerate(uvs):
                                for it in range(NI):
                                    nc.tensor.matmul(
                                        pss[k][:],
                                        wmod[it][:, uv, o0 : o0 + P],
                                        vt[it][:, uv].rearrange(
                                            "p t c -> p (t c)"),
                                        start=(it == 0),
                                        stop=(it == NI - 1),
                                    )
                            for k, uv in enumerate(uvs):
                                nc.scalar.activation(
                                    msb[:, uv, :], pss[k][:],
                                    AF.Copy, scale=dsb[:, ot : ot + 1],
                                )
                            # output transform pass1 for this phase's v's
                            # AT = [[1,1,1,0],[0,1,-1,-1]]
                            for v in vset:
                                p0 = psb[:, v, :]
                                p1 = psb[:, 4 + v, :]
                                tt(p0, M[0 * 4 + v], M[1 * 4 + v], ALU.add)
                                tt(p0, p0, M[2 * 4 + v], ALU.add)
                                tt(p1, M[1 * 4 + v], M[2 * 4 + v],
                                   ALU.subtract)
                                tt(p1, p1, M[3 * 4 + v], ALU.subtract)
                        osb = osbp.tile([P, TH, 2, T, 2], F32, tag="osb")
                        for a in range(2):
                            pa = [
                                psb[:, a * 4 + v, :].rearrange(
                                    "p (t c) -> p t c", c=T)
                                for v in range(4)
                            ]
                            yr = [osb[:, :, a, :, b] for b in range(2)]
                            tt(yr[0], pa[0], pa[1], ALU.add)
                            tt(yr[0], yr[0], pa[2], ALU.add)
                            tt(yr[1], pa[1], pa[2], ALU.subtract)
                            tt(yr[1], yr[1], pa[3], ALU.subtract)
                        nc.sync.dma_start(
                            out[o0 : o0 + P, 2 * TH * h : 2 * TH * (h + 1), :],
                            osb[:].rearrange("p t a c b -> p (t a) (c b)"),
                        )
    nc.compile()
    return nc


_NC_CACHE = None


def _get_nc():
    global _NC_CACHE
    if _NC_CACHE is None:
        _NC_CACHE = build_nc()
    return _NC_CACHE


_G = np.array(
    [[1, 0, 0], [0.5, 0.5, 0.5], [0.5, -0.5, 0.5], [0, 0, 1]], np.float64
)


def make_in_maps(img, s, weight):
    img = np.asarray(img, dtype=np.float32)
    s = np.ascontiguousarray(np.asarray(s, dtype=np.float32))
    weight = np.asarray(weight, dtype=np.float32)
    # zero-pad, then DE-INTERLEAVE x: [even cols | odd cols]
    imgp = np.zeros((img.shape[0], NI, P, HP, WP), dtype=np.float32)
    imgp[:, :, :, 1 : H + 1, 1 : W + 1] = img.reshape(-1, NI, P, H, W)
    imgd = np.concatenate(
        [imgp[..., 0::2], imgp[..., 1::2]], axis=-1
    ).astype(ml_dtypes.bfloat16)
    # wg[u,v,i,o] = (G W G^T)[o,i,u,v] -> [NI, P, (u,v,o)]
    wgf = np.einsum("ua,oiab,vb->uvio", _G, weight.astype(np.float64), _G)
    wgt = np.ascontiguousarray(
        wgf.transpose(2, 0, 1, 3)          # [i, u, v, o]
        .reshape(NI, P, 16 * COUT)
        .astype(ml_dtypes.bfloat16)
    )
    at = np.ascontiguousarray(
        (weight.astype(np.float64) ** 2)
        .sum(axis=(2, 3))
        .T.astype(ml_dtypes.bfloat16)
    )
    return [
        {"img": imgd[b], "s": s[b], "wg": wgt, "at": at} for b in range(N_CORES)
    ]


def kernel(img, s, weight):
    nc = _get_nc()
    in_maps = make_in_maps(img, s, weight)
    res = run_bass_kernel_spmd(nc, in_maps, list(range(N_CORES)))
    return np.stack([res.results[b]["out"] for b in range(N_CORES)], axis=0)
